# revision 18
# baseline (speedup 1.0000x reference)
"""Chamfer loss kernel for TRN2 (8 NeuronCores, data-parallel over batch).

Reference computation (per batch b):
  t = l2_normalize(tokens[b])      # (K=1024, D=128)
  i = l2_normalize(interests[b])   # (M=64,  D=128)
  dist[k,m] = sqrt(2 - 2*dot(t_k, i_m))   (unit vectors)
  loss = mean_bm(min_k dist) + 0.3 * mean_bk(min_m dist)

Design notes (per core, 64 batches; engine-balanced against the ~1.46us/batch
token-DMA floor):
  - interests pre-normalized once (phase 0, bn_stats for sum-of-squares),
    kept transposed in SBUF as bf16 iT_all.
  - per batch:
      DMA  tokens[b] -> t_all fp32 [128,(8n),128d]
      PE   8 transposes (fp32) -> psum; ACT evacuates psum -> tT bf16
      DVE  tq = tT*tT (bf16 2x mode)
      PE   8 dot matmuls   pdots[k,(n m)] = tT_n.T @ iT_b        (bf16)
      PE   8 ones-column matmuls sums[k,n] = tq_n.T @ ones  == sum_d t^2
           (lands token sum-of-squares directly in [k-partition, n] layout:
            no partition reduce, no DMA gather)
      ACT  tnrm = sqrt(sums) from psum;  DVE invt = 1/tnrm
      DVE  dn = pdots * invt  (fused normalize + psum evacuation, bf16)
      POOL st_t_all[:,b,:] = max_m dn   (deferred sqrt, once at the end)
      POOL m2 = max(dn[:,0:4,:], dn[:,4:8,:]);  DVE m3, nmax (bf16 tree)
      POOL partition-max nmax -> st_i chunk;  every 8 batches ACT applies
           sqrt(2-2x) to the chunk and POOL accumulates.
Host combines the 8 per-core partial sums.
"""

import numpy as np
from contextlib import ExitStack

import concourse.bass as bass
import concourse.bass_isa as bass_isa
import concourse.mybir as mybir
import concourse.tile as tile
from concourse import bacc
from concourse.bass_utils import run_bass_kernel_spmd

N_CORES = 8
B, K, M, D = 512, 1024, 64, 128
B_LOC = B // N_CORES          # 64 batches per core
KT = K // 128                 # 8 token tiles of [128, D] per batch
NG = B_LOC * M // 128         # 32 interest row-groups of 128
ALPHA_T_TO_I = 0.3
SI = 8                        # i-side sqrt staging (batches per chunk)
LAG = 5

F32 = mybir.dt.float32
BF16 = mybir.dt.bfloat16
AX = mybir.AxisListType
OP = mybir.AluOpType
ACT = mybir.ActivationFunctionType
RED = bass_isa.ReduceOp


def build(b_loc=B_LOC):
    assert b_loc % SI == 0
    nc = bacc.Bacc(
        "TRN2",
        target_bir_lowering=False,
        debug=False,
        num_devices=N_CORES,
    )
    tokens = nc.dram_tensor("tokens", [b_loc, K, D], F32, kind="ExternalInput").ap()
    interests = nc.dram_tensor(
        "interests", [b_loc, M, D], F32, kind="ExternalInput"
    ).ap()
    out = nc.dram_tensor("out", [1, 2], F32, kind="ExternalOutput").ap()

    with ExitStack() as ctx:
        tc = ctx.enter_context(tile.TileContext(nc))
        singles = ctx.enter_context(tc.tile_pool(name="singles", bufs=1))
        tok_pool = ctx.enter_context(tc.tile_pool(name="tok", bufs=4))
        tT_pool = ctx.enter_context(tc.tile_pool(name="tT", bufs=4))
        tq_pool = ctx.enter_context(tc.tile_pool(name="tq", bufs=4))
        dn_pool = ctx.enter_context(tc.tile_pool(name="dn", bufs=4))
        m2_pool = ctx.enter_context(tc.tile_pool(name="m2", bufs=4))
        small = ctx.enter_context(tc.tile_pool(name="small", bufs=16))
        stage = ctx.enter_context(tc.tile_pool(name="stage", bufs=3))
        p_tT = ctx.enter_context(tc.tile_pool(name="p_tT", bufs=3, space="PSUM"))
        p_dots = ctx.enter_context(tc.tile_pool(name="p_dots", bufs=2, space="PSUM"))
        p_sums = ctx.enter_context(tc.tile_pool(name="p_sums", bufs=1, space="PSUM"))

        identity = singles.tile([128, 128], F32)
        nc.gpsimd.memset(identity, 0.0)
        nc.gpsimd.affine_select(
            out=identity, in_=identity, compare_op=OP.not_equal, fill=1.0,
            base=0, pattern=[[-1, 128]], channel_multiplier=1,
        )
        ones_bf = singles.tile([128, 1], BF16)
        nc.vector.memset(ones_bf, 1.0)
        two = singles.tile([128, 1], F32)
        nc.vector.memset(two, 2.0)
        st_t_all = singles.tile([128, b_loc, KT], BF16)
        acc_i = singles.tile([128, SI * M], F32)
        nc.vector.memset(acc_i, 0.0)

        # ---------- phase 0: normalize + transpose all interests ----------
        # Emitted in 4 chunks, interleaved into the main pipeline so the
        # first token batches' DMA/transpose/square stages are not serialized
        # behind the whole interests preparation.
        i_flat = interests.rearrange("b m d -> (b m) d").rearrange(
            "(g p) d -> p g d", p=128
        )  # [128, NG, 128]
        i_all = singles.tile([128, NG, D], F32)
        i_n = singles.tile([128, NG, D], F32)
        iT_all = singles.tile([128, NG, 128], BF16)  # [d, (g, row)]
        NCH = 4
        GC = NG // NCH  # 8 groups per chunk

        def pA(c):  # interests chunk DMA
            sl = slice(GC * c, GC * (c + 1))
            nc.sync.dma_start(out=i_all[:, sl], in_=i_flat[:, sl])

        def pB(c):  # sum-of-squares per interest row
            sl = slice(GC * c, GC * (c + 1))
            isq = tq_pool.tile([128, GC, D], BF16, tag="isq")
            nc.scalar.square(isq, i_all[:, sl])
            issq = small.tile([128, GC], F32, tag="issq")
            nc.vector.tensor_reduce(issq, isq, axis=AX.X, op=OP.add)
            inrm = small.tile([128, GC], F32, tag="inrm")
            nc.scalar.sqrt(inrm, issq)
            invi = small.tile([128, GC], F32, tag="invi")
            nc.vector.reciprocal(invi, inrm)
            nc.gpsimd.tensor_mul(
                i_n[:, sl], i_all[:, sl], invi.broadcast_to([128, GC, D])
            )

        def pC(c):  # transpose + evacuate chunk
            for cc in range(GC * c, GC * (c + 1), 4):
                piT = p_tT.tile([128, 4, 128], F32, tag="ptT")
                for j in range(4):
                    nc.tensor.transpose(
                        piT[:, j, :], i_n[:, cc + j, :], identity
                    )
                dst = iT_all[:, cc:cc + 4, :].rearrange("p a b -> p (a b)")
                src = piT[:, :4, :].rearrange("p a b -> p (a b)")
                nc.scalar.copy(dst, src)

        def iT_of(b):
            return iT_all[:, b // 2, (b % 2) * M:(b % 2) * M + M]

        # ---------- software-pipelined main loop ----------
        # Post-matmul vector work is fused over batch PAIRS to amortize the
        # fixed per-op access latencies on DVE.  All free-axis reductions and
        # maxes are DVE-only (gpsimd has neither); Pool gets the elementwise
        # square's other half, the partition-max, and the accumulate adds.
        t_of, tT_of, pd_of, ps_of, iv_of = {}, {}, {}, {}, {}
        m2_of, sti_of = {}, {}

        def s0(b):  # token DMA
            t_all = tok_pool.tile([128, KT, D], F32)
            nc.sync.dma_start(
                out=t_all, in_=tokens[b].rearrange("(n p) d -> p n d", p=128)
            )
            t_of[b] = t_all

        def s1(b):  # transposes + evacuation (fp32 psum -> bf16 sbuf)
            t_all = t_of.pop(b)
            tT = tT_pool.tile([128, KT, 128], BF16, tag="tT")
            for h in range(2):
                ptT = p_tT.tile([128, KT // 2, 128], F32, tag="ptT")
                for j in range(KT // 2):
                    nc.tensor.transpose(
                        ptT[:, j, :], t_all[:, 4 * h + j, :], identity
                    )
                nc.scalar.copy(
                    tT[:, 4 * h:4 * h + 4, :].rearrange("p a b -> p (a b)"),
                    ptT.rearrange("p a b -> p (a b)"),
                )
            tT_of[b] = tT

        def s2(b):  # squares (DVE/Pool halves), dots, sum-of-squares columns
            tT = tT_of.pop(b)
            tq = tq_pool.tile([128, KT, 128], BF16, tag="tq")
            nc.vector.tensor_mul(
                tq[:, 0:4, :].rearrange("p a b -> p (a b)"),
                tT[:, 0:4, :].rearrange("p a b -> p (a b)"),
                tT[:, 0:4, :].rearrange("p a b -> p (a b)"),
            )
            nc.gpsimd.tensor_mul(
                tq[:, 4:8, :].rearrange("p a b -> p (a b)"),
                tT[:, 4:8, :].rearrange("p a b -> p (a b)"),
                tT[:, 4:8, :].rearrange("p a b -> p (a b)"),
            )
            if b % 2 == 0:
                pd2 = p_dots.tile([128, 2, KT, M], F32, tag="pd")
                ps2 = p_sums.tile([128, 2, KT], F32, tag="ps")
                pd_of[b // 2] = pd2
                ps_of[b // 2] = ps2
            else:
                pd2 = pd_of[b // 2]
                ps2 = ps_of[b // 2]
            h = b % 2
            iT = iT_of(b)
            for n in range(KT):
                nc.tensor.matmul(
                    ps2[:, h, n:n + 1], lhsT=tq[:, n, :], rhs=ones_bf,
                    start=True, stop=True,
                )
            for n in range(KT):
                nc.tensor.matmul(
                    pd2[:, h, n, :], lhsT=tT[:, n, :], rhs=iT,
                    start=True, stop=True,
                )

        def s3(j):  # token norms for pair j: invt = sqrt(1/s)
            # reciprocal first (DVE, reads psum straight from PE — no ACT
            # dependency on the DVE critical path), then sqrt on ACT.
            rs = small.tile([128, 2, KT], F32, tag="rs")
            nc.vector.reciprocal(rs, ps_of.pop(j))
            invt = small.tile([128, 2, KT], F32, tag="invt")
            nc.scalar.sqrt(invt, rs)
            iv_of[j] = invt

        q5_of, m2q_of = {}, {}

        def s4(j):  # normalize; m-tree levels 1-2; n-tree level 1  (pair j)
            pd2 = pd_of.pop(j)
            invt = iv_of.pop(j)
            q = j // 2
            h = j % 2
            if h == 0:
                q5_new = m2_pool.tile([128, 2, 2, KT, 16], BF16, tag="q5")
                m2q_new = m2_pool.tile([128, 2, 2, KT // 2, M], BF16, tag="m2q")
                q5_of[q] = q5_new
                m2q_of[q] = m2q_new
            dn = dn_pool.tile([128, 2, KT, M], BF16, tag="dn")
            nc.vector.tensor_mul(dn, pd2, invt.broadcast_to([128, 2, KT, M]))
            # t->i: per-token max over m, bf16 TT-max tree levels 64->32->16
            t32 = m2_pool.tile([128, 2, KT, 32], BF16, tag="t32")
            nc.vector.tensor_max(t32, dn[:, :, :, 0:32], dn[:, :, :, 32:64])
            nc.vector.tensor_max(
                q5_of[q][:, h], t32[:, :, :, 0:16], t32[:, :, :, 16:32]
            )
            # i->t: max over n, tree level 1 (8 -> 4)
            nc.vector.tensor_max(
                m2q_of[q][:, h], dn[:, :, 0:KT // 2, :], dn[:, :, KT // 2:KT, :]
            )

        def s5(q):  # finish both reductions for quad q (4 batches)
            b0 = 4 * q
            s2i = b0 % SI
            g = b0 // SI
            if s2i == 0:
                st_i_new = stage.tile([128, SI, M], BF16, tag="sti")
                sti_of[g] = st_i_new
            st_i = sti_of[g]
            # t->i tail: 16 -> 8 -> 4 -> 2 -> 1 over m
            q5 = q5_of.pop(q)
            t8 = small.tile([128, 2, 2, KT, 8], BF16, tag="t8")
            nc.vector.tensor_max(t8, q5[:, :, :, :, 0:8], q5[:, :, :, :, 8:16])
            t4 = small.tile([128, 2, 2, KT, 4], BF16, tag="t4")
            nc.vector.tensor_max(t4, t8[:, :, :, :, 0:4], t8[:, :, :, :, 4:8])
            t2 = small.tile([128, 2, 2, KT, 2], BF16, tag="t2")
            nc.vector.tensor_max(t2, t4[:, :, :, :, 0:2], t4[:, :, :, :, 2:4])
            nc.vector.tensor_max(
                st_t_all[:, b0:b0 + 4, :].rearrange(
                    "p (a b) (c o) -> p a b c o", a=2, o=1
                ),
                t2[:, :, :, :, 0:1], t2[:, :, :, :, 1:2],
            )
            # i->t tail: n-tree levels 2-3, then partition max
            m2q = m2q_of.pop(q)
            m3 = small.tile([128, 2, 2, 2, M], BF16, tag="m3")
            nc.vector.tensor_max(m3, m2q[:, :, :, 0:2, :], m2q[:, :, :, 2:4, :])
            nm2 = small.tile([128, 2, 2, M], BF16, tag="nm2")
            nc.vector.tensor_max(nm2, m3[:, :, :, 0, :], m3[:, :, :, 1, :])
            nc.gpsimd.partition_all_reduce(
                st_i[:, s2i:s2i + 4, :].rearrange("p a b -> p (a b)"),
                nm2.rearrange("p a b c -> p (a b c)"),
                channels=128, reduce_op=RED.max,
            )
            if s2i == SI - 4:
                del sti_of[g]
                di = stage.tile([128, SI * M], BF16, tag="di")
                nc.scalar.activation(
                    di, st_i.rearrange("p a b -> p (a b)"),
                    ACT.Sqrt, bias=two[:], scale=-2.0,
                )
                nc.gpsimd.tensor_add(acc_i, acc_i, di)

        nj = b_loc // 2
        nq = b_loc // 4
        for v in range(b_loc + 2 * LAG + 2):
            # interleaved phase-0 chunks
            if v % 2 == 0 and v // 2 < NCH:
                pA(v // 2)
            if v % 2 == 1 and v // 2 < NCH:
                pB(v // 2)
            if v >= 2 and v % 2 == 0 and (v - 2) // 2 < NCH:
                pC((v - 2) // 2)
            # quad stages
            if v >= 8 and (v - 8) % 4 == 0 and (v - 8) // 4 < nq:
                s5((v - 8) // 4)
            # pair stages
            if v >= 4 and (v - 4) % 2 == 1 and (v - 4) // 2 < nj:
                s4((v - 4) // 2)
            if v >= 3 and (v - 3) % 2 == 1 and (v - 3) // 2 < nj:
                s3((v - 3) // 2)
            # batch stages
            if v >= 2 and v - 2 < b_loc:
                s2(v - 2)
            if v >= 1 and v - 1 < b_loc:
                s1(v - 1)
            if v < b_loc:
                s0(v)

        # ---------- final reductions ----------
        dt = singles.tile([128, b_loc * KT], BF16)
        nc.scalar.activation(
            dt, st_t_all.rearrange("p a b -> p (a b)"),
            ACT.Sqrt, bias=two[:], scale=-2.0,
        )
        red_t = singles.tile([128, 1], F32)
        nc.vector.tensor_reduce(red_t, dt, axis=AX.X, op=OP.add)
        rep_t = singles.tile([128, 1], F32)
        nc.gpsimd.partition_all_reduce(
            rep_t, red_t, channels=128, reduce_op=RED.add
        )
        red_i = singles.tile([128, 1], F32)
        nc.vector.tensor_reduce(red_i, acc_i, axis=AX.X, op=OP.add)
        out_sb = small.tile([1, 2], F32, tag="out_sb")
        nc.scalar.copy(out_sb[:, 0:1], rep_t[0:1, :])
        nc.scalar.copy(out_sb[:, 1:2], red_i[0:1, :])
        nc.sync.dma_start(out=out, in_=out_sb)

    nc.compile()
    return nc


_NC_CACHE = None


def _get_nc():
    global _NC_CACHE
    if _NC_CACHE is None:
        _NC_CACHE = build()
    return _NC_CACHE


def kernel(tokens: np.ndarray, interests: np.ndarray, _trace=False) -> np.ndarray:
    tokens = np.ascontiguousarray(tokens, dtype=np.float32)
    interests = np.ascontiguousarray(interests, dtype=np.float32)
    assert tokens.shape == (B, K, D) and interests.shape == (B, M, D)

    nc = _get_nc()
    in_maps = [
        {
            "tokens": tokens[c * B_LOC:(c + 1) * B_LOC],
            "interests": interests[c * B_LOC:(c + 1) * B_LOC],
        }
        for c in range(N_CORES)
    ]
    res = run_bass_kernel_spmd(
        nc, in_maps, core_ids=list(range(N_CORES)), trace=_trace
    )
    sum_t = 0.0  # sum over all (b, k) of min_m dist
    sum_i = 0.0  # sum over all (b, m) of min_k dist
    for r in res.results:
        sum_t += float(r["out"][0, 0])
        sum_i += float(r["out"][0, 1])
    loss = sum_i / (B * M) + ALPHA_T_TO_I * sum_t / (B * K)
    kernel.last_results = res
    return np.array(loss, dtype=np.float32)


# revision 21
# speedup vs baseline: 1.0356x; 1.0356x over previous
"""Chamfer loss kernel for TRN2 (8 NeuronCores, data-parallel over batch).

Reference computation (per batch b):
  t = l2_normalize(tokens[b])      # (K=1024, D=128)
  i = l2_normalize(interests[b])   # (M=64,  D=128)
  dist[k,m] = sqrt(2 - 2*dot(t_k, i_m))   (unit vectors)
  loss = mean_bm(min_k dist) + 0.3 * mean_bk(min_m dist)

Design notes (per core, 64 batches; engine-balanced against the ~1.46us/batch
token-DMA floor):
  - interests pre-normalized once (phase 0, bn_stats for sum-of-squares),
    kept transposed in SBUF as bf16 iT_all.
  - per batch:
      DMA  tokens[b] -> t_all fp32 [128,(8n),128d]
      PE   8 transposes (fp32) -> psum; ACT evacuates psum -> tT bf16
      DVE  tq = tT*tT (bf16 2x mode)
      PE   8 dot matmuls   pdots[k,(n m)] = tT_n.T @ iT_b        (bf16)
      PE   8 ones-column matmuls sums[k,n] = tq_n.T @ ones  == sum_d t^2
           (lands token sum-of-squares directly in [k-partition, n] layout:
            no partition reduce, no DMA gather)
      ACT  tnrm = sqrt(sums) from psum;  DVE invt = 1/tnrm
      DVE  dn = pdots * invt  (fused normalize + psum evacuation, bf16)
      POOL st_t_all[:,b,:] = max_m dn   (deferred sqrt, once at the end)
      POOL m2 = max(dn[:,0:4,:], dn[:,4:8,:]);  DVE m3, nmax (bf16 tree)
      POOL partition-max nmax -> st_i chunk;  every 8 batches ACT applies
           sqrt(2-2x) to the chunk and POOL accumulates.
Host combines the 8 per-core partial sums.
"""

import numpy as np
from contextlib import ExitStack

import concourse.bass as bass
import concourse.bass_isa as bass_isa
import concourse.mybir as mybir
import concourse.tile as tile
from concourse import bacc
from concourse.bass_utils import run_bass_kernel_spmd

N_CORES = 8
B, K, M, D = 512, 1024, 64, 128
B_LOC = B // N_CORES          # 64 batches per core
KT = K // 128                 # 8 token tiles of [128, D] per batch
NG = B_LOC * M // 128         # 32 interest row-groups of 128
ALPHA_T_TO_I = 0.3
SI = 8                        # i-side sqrt staging (batches per chunk)
LAG = 5

F32 = mybir.dt.float32
BF16 = mybir.dt.bfloat16
AX = mybir.AxisListType
OP = mybir.AluOpType
ACT = mybir.ActivationFunctionType
RED = bass_isa.ReduceOp


def build(b_loc=B_LOC):
    assert b_loc % SI == 0
    nc = bacc.Bacc(
        "TRN2",
        target_bir_lowering=False,
        debug=False,
        num_devices=N_CORES,
    )
    tokens = nc.dram_tensor("tokens", [b_loc, K, D], F32, kind="ExternalInput").ap()
    interests = nc.dram_tensor(
        "interests", [b_loc, M, D], F32, kind="ExternalInput"
    ).ap()
    out = nc.dram_tensor("out", [1, 2], F32, kind="ExternalOutput").ap()

    with ExitStack() as ctx:
        tc = ctx.enter_context(tile.TileContext(nc))
        singles = ctx.enter_context(tc.tile_pool(name="singles", bufs=1))
        tok_pool = ctx.enter_context(tc.tile_pool(name="tok", bufs=4))
        tT_pool = ctx.enter_context(tc.tile_pool(name="tT", bufs=4))
        tq_pool = ctx.enter_context(tc.tile_pool(name="tq", bufs=4))
        dn_pool = ctx.enter_context(tc.tile_pool(name="dn", bufs=4))
        m2_pool = ctx.enter_context(tc.tile_pool(name="m2", bufs=4))
        small = ctx.enter_context(tc.tile_pool(name="small", bufs=16))
        stage = ctx.enter_context(tc.tile_pool(name="stage", bufs=3))
        p_tT = ctx.enter_context(tc.tile_pool(name="p_tT", bufs=3, space="PSUM"))
        p_dots = ctx.enter_context(tc.tile_pool(name="p_dots", bufs=2, space="PSUM"))
        p_sums = ctx.enter_context(tc.tile_pool(name="p_sums", bufs=1, space="PSUM"))

        identity = singles.tile([128, 128], F32)
        nc.gpsimd.memset(identity, 0.0)
        nc.gpsimd.affine_select(
            out=identity, in_=identity, compare_op=OP.not_equal, fill=1.0,
            base=0, pattern=[[-1, 128]], channel_multiplier=1,
        )
        ones_bf = singles.tile([128, 1], BF16)
        nc.vector.memset(ones_bf, 1.0)
        two = singles.tile([128, 1], F32)
        nc.vector.memset(two, 2.0)
        st_t_all = singles.tile([128, b_loc, KT], BF16)
        acc_i = singles.tile([128, SI * M], F32)
        nc.vector.memset(acc_i, 0.0)

        # ---------- phase 0: normalize + transpose all interests ----------
        # Emitted in 4 chunks, interleaved into the main pipeline so the
        # first token batches' DMA/transpose/square stages are not serialized
        # behind the whole interests preparation.
        i_flat = interests.rearrange("b m d -> (b m) d").rearrange(
            "(g p) d -> p g d", p=128
        )  # [128, NG, 128]
        i_all = singles.tile([128, NG, D], F32)
        i_n = singles.tile([128, NG, D], F32)
        iT_all = singles.tile([128, NG, 128], BF16)  # [d, (g, row)]
        NCH = 4
        GC = NG // NCH  # 8 groups per chunk

        def pA(c):  # interests chunk DMA
            sl = slice(GC * c, GC * (c + 1))
            nc.sync.dma_start(out=i_all[:, sl], in_=i_flat[:, sl])

        def pB(c):  # sum-of-squares per interest row
            sl = slice(GC * c, GC * (c + 1))
            isq = tq_pool.tile([128, GC, D], BF16, tag="isq")
            nc.scalar.square(isq, i_all[:, sl])
            issq = small.tile([128, GC], F32, tag="issq")
            nc.vector.tensor_reduce(issq, isq, axis=AX.X, op=OP.add)
            inrm = small.tile([128, GC], F32, tag="inrm")
            nc.scalar.sqrt(inrm, issq)
            invi = small.tile([128, GC], F32, tag="invi")
            nc.vector.reciprocal(invi, inrm)
            nc.gpsimd.tensor_mul(
                i_n[:, sl], i_all[:, sl], invi.broadcast_to([128, GC, D])
            )

        def pC(c):  # transpose + evacuate chunk
            for cc in range(GC * c, GC * (c + 1), 4):
                piT = p_tT.tile([128, 4, 128], F32, tag="ptT")
                for j in range(4):
                    nc.tensor.transpose(
                        piT[:, j, :], i_n[:, cc + j, :], identity
                    )
                dst = iT_all[:, cc:cc + 4, :].rearrange("p a b -> p (a b)")
                src = piT[:, :4, :].rearrange("p a b -> p (a b)")
                nc.scalar.copy(dst, src)

        def iT_of(b):
            return iT_all[:, b // 2, (b % 2) * M:(b % 2) * M + M]

        # ---------- software-pipelined main loop ----------
        # Post-matmul vector work is fused over batch PAIRS to amortize the
        # fixed per-op access latencies on DVE.  All free-axis reductions and
        # maxes are DVE-only (gpsimd has neither); Pool gets the elementwise
        # square's other half, the partition-max, and the accumulate adds.
        t_of, tT_of, pd_of, ps_of, iv_of = {}, {}, {}, {}, {}
        m2_of, sti_of = {}, {}

        def s0(b):  # token DMA
            t_all = tok_pool.tile([128, KT, D], F32)
            nc.sync.dma_start(
                out=t_all, in_=tokens[b].rearrange("(n p) d -> p n d", p=128)
            )
            t_of[b] = t_all

        def s1(b):  # transposes + evacuation (fp32 psum -> bf16 sbuf)
            t_all = t_of.pop(b)
            tT = tT_pool.tile([128, KT, 128], BF16, tag="tT")
            for h in range(2):
                ptT = p_tT.tile([128, KT // 2, 128], F32, tag="ptT")
                for j in range(KT // 2):
                    nc.tensor.transpose(
                        ptT[:, j, :], t_all[:, 4 * h + j, :], identity
                    )
                nc.scalar.copy(
                    tT[:, 4 * h:4 * h + 4, :].rearrange("p a b -> p (a b)"),
                    ptT.rearrange("p a b -> p (a b)"),
                )
            tT_of[b] = tT

        def s2(b):  # squares (DVE/Pool halves), dots, sum-of-squares columns
            tT = tT_of.pop(b)
            tq = tq_pool.tile([128, KT, 128], BF16, tag="tq")
            nc.vector.tensor_mul(
                tq[:, 0:4, :].rearrange("p a b -> p (a b)"),
                tT[:, 0:4, :].rearrange("p a b -> p (a b)"),
                tT[:, 0:4, :].rearrange("p a b -> p (a b)"),
            )
            nc.gpsimd.tensor_mul(
                tq[:, 4:8, :].rearrange("p a b -> p (a b)"),
                tT[:, 4:8, :].rearrange("p a b -> p (a b)"),
                tT[:, 4:8, :].rearrange("p a b -> p (a b)"),
            )
            if b % 2 == 0:
                pd2 = p_dots.tile([128, 2, KT, M], F32, tag="pd")
                ps2 = p_sums.tile([128, 2, KT], F32, tag="ps")
                pd_of[b // 2] = pd2
                ps_of[b // 2] = ps2
            else:
                pd2 = pd_of[b // 2]
                ps2 = ps_of[b // 2]
            h = b % 2
            iT = iT_of(b)
            for n in range(KT):
                nc.tensor.matmul(
                    ps2[:, h, n:n + 1], lhsT=tq[:, n, :], rhs=ones_bf,
                    start=True, stop=True,
                )
            for n in range(KT):
                nc.tensor.matmul(
                    pd2[:, h, n, :], lhsT=tT[:, n, :], rhs=iT,
                    start=True, stop=True,
                )

        tn_of = {}

        def s3a(b):  # per-batch sqrt of sum-of-squares (ACT, reads psum)
            j = b // 2
            h = b % 2
            if h == 0:
                tn_new = small.tile([128, 2, KT], F32, tag="tnrm")
                tn_of[j] = tn_new
            nc.scalar.sqrt(tn_of[j][:, h], ps_of[j][:, h])
            if h == 1:
                del ps_of[j]

        def s3(j):  # pair reciprocal -> invt
            invt = small.tile([128, 2, KT], F32, tag="invt")
            nc.vector.reciprocal(invt, tn_of.pop(j))
            iv_of[j] = invt

        q5_of, m2q_of = {}, {}

        def s4(j):  # normalize; m-tree levels 1-2; n-tree level 1  (pair j)
            pd2 = pd_of.pop(j)
            invt = iv_of.pop(j)
            q = j // 2
            h = j % 2
            if h == 0:
                q5_new = m2_pool.tile([128, 2, 2, KT, 16], BF16, tag="q5")
                m2q_new = m2_pool.tile([128, 2, 2, KT // 2, M], BF16, tag="m2q")
                q5_of[q] = q5_new
                m2q_of[q] = m2q_new
            dn = dn_pool.tile([128, 2, KT, M], BF16, tag="dn")
            nc.vector.tensor_mul(dn, pd2, invt.broadcast_to([128, 2, KT, M]))
            # t->i: per-token max over m, bf16 TT-max tree levels 64->32->16
            t32 = m2_pool.tile([128, 2, KT, 32], BF16, tag="t32")
            nc.vector.tensor_max(t32, dn[:, :, :, 0:32], dn[:, :, :, 32:64])
            nc.vector.tensor_max(
                q5_of[q][:, h], t32[:, :, :, 0:16], t32[:, :, :, 16:32]
            )
            # i->t: max over n, tree level 1 (8 -> 4)
            nc.vector.tensor_max(
                m2q_of[q][:, h], dn[:, :, 0:KT // 2, :], dn[:, :, KT // 2:KT, :]
            )

        def s5(q):  # finish both reductions for quad q (4 batches)
            b0 = 4 * q
            s2i = b0 % SI
            g = b0 // SI
            if s2i == 0:
                st_i_new = stage.tile([128, SI, M], BF16, tag="sti")
                sti_of[g] = st_i_new
            st_i = sti_of[g]
            # t->i tail: 16 -> 8 -> 4 -> 2 -> 1 over m
            q5 = q5_of.pop(q)
            t8 = small.tile([128, 2, 2, KT, 8], BF16, tag="t8")
            nc.vector.tensor_max(t8, q5[:, :, :, :, 0:8], q5[:, :, :, :, 8:16])
            t4 = small.tile([128, 2, 2, KT, 4], BF16, tag="t4")
            nc.vector.tensor_max(t4, t8[:, :, :, :, 0:4], t8[:, :, :, :, 4:8])
            t2 = small.tile([128, 2, 2, KT, 2], BF16, tag="t2")
            nc.vector.tensor_max(t2, t4[:, :, :, :, 0:2], t4[:, :, :, :, 2:4])
            nc.vector.tensor_max(
                st_t_all[:, b0:b0 + 4, :].rearrange(
                    "p (a b) (c o) -> p a b c o", a=2, o=1
                ),
                t2[:, :, :, :, 0:1], t2[:, :, :, :, 1:2],
            )
            # i->t tail: n-tree levels 2-3, then partition max
            m2q = m2q_of.pop(q)
            m3 = small.tile([128, 2, 2, 2, M], BF16, tag="m3")
            nc.vector.tensor_max(m3, m2q[:, :, :, 0:2, :], m2q[:, :, :, 2:4, :])
            nm2 = small.tile([128, 2, 2, M], BF16, tag="nm2")
            nc.vector.tensor_max(nm2, m3[:, :, :, 0, :], m3[:, :, :, 1, :])
            nc.gpsimd.partition_all_reduce(
                st_i[:, s2i:s2i + 4, :].rearrange("p a b -> p (a b)"),
                nm2.rearrange("p a b c -> p (a b c)"),
                channels=128, reduce_op=RED.max,
            )
            if s2i == SI - 4:
                del sti_of[g]
                di = stage.tile([128, SI * M], BF16, tag="di")
                nc.scalar.activation(
                    di, st_i.rearrange("p a b -> p (a b)"),
                    ACT.Sqrt, bias=two[:], scale=-2.0,
                )
                nc.gpsimd.tensor_add(acc_i, acc_i, di)

        nj = b_loc // 2
        nq = b_loc // 4
        for v in range(b_loc + 2 * LAG + 2):
            # interleaved phase-0 chunks
            if v % 2 == 0 and v // 2 < NCH:
                pA(v // 2)
            if v % 2 == 1 and v // 2 < NCH:
                pB(v // 2)
            if v >= 2 and v % 2 == 0 and (v - 2) // 2 < NCH:
                pC((v - 2) // 2)
            # quad stages
            if v >= 8 and (v - 8) % 4 == 0 and (v - 8) // 4 < nq:
                s5((v - 8) // 4)
            # pair stages (recip must precede D4 in DVE program order)
            if v >= 5 and (v - 5) % 2 == 0 and (v - 5) // 2 < nj:
                s3((v - 5) // 2)
            if v >= 4 and (v - 4) % 2 == 1 and (v - 4) // 2 < nj:
                s4((v - 4) // 2)
            if v >= 3 and v - 3 < b_loc:
                s3a(v - 3)
            # batch stages
            if v >= 2 and v - 2 < b_loc:
                s2(v - 2)
            if v >= 1 and v - 1 < b_loc:
                s1(v - 1)
            if v < b_loc:
                s0(v)

        # ---------- final reductions ----------
        dt = singles.tile([128, b_loc * KT], BF16)
        nc.scalar.activation(
            dt, st_t_all.rearrange("p a b -> p (a b)"),
            ACT.Sqrt, bias=two[:], scale=-2.0,
        )
        red_t = singles.tile([128, 1], F32)
        nc.vector.tensor_reduce(red_t, dt, axis=AX.X, op=OP.add)
        rep_t = singles.tile([128, 1], F32)
        nc.gpsimd.partition_all_reduce(
            rep_t, red_t, channels=128, reduce_op=RED.add
        )
        red_i = singles.tile([128, 1], F32)
        nc.vector.tensor_reduce(red_i, acc_i, axis=AX.X, op=OP.add)
        out_sb = small.tile([1, 2], F32, tag="out_sb")
        nc.scalar.copy(out_sb[:, 0:1], rep_t[0:1, :])
        nc.scalar.copy(out_sb[:, 1:2], red_i[0:1, :])
        nc.sync.dma_start(out=out, in_=out_sb)

    nc.compile()
    return nc


_NC_CACHE = None


def _get_nc():
    global _NC_CACHE
    if _NC_CACHE is None:
        _NC_CACHE = build()
    return _NC_CACHE


def kernel(tokens: np.ndarray, interests: np.ndarray, _trace=False) -> np.ndarray:
    tokens = np.ascontiguousarray(tokens, dtype=np.float32)
    interests = np.ascontiguousarray(interests, dtype=np.float32)
    assert tokens.shape == (B, K, D) and interests.shape == (B, M, D)

    nc = _get_nc()
    in_maps = [
        {
            "tokens": tokens[c * B_LOC:(c + 1) * B_LOC],
            "interests": interests[c * B_LOC:(c + 1) * B_LOC],
        }
        for c in range(N_CORES)
    ]
    res = run_bass_kernel_spmd(
        nc, in_maps, core_ids=list(range(N_CORES)), trace=_trace
    )
    sum_t = 0.0  # sum over all (b, k) of min_m dist
    sum_i = 0.0  # sum over all (b, m) of min_k dist
    for r in res.results:
        sum_t += float(r["out"][0, 0])
        sum_i += float(r["out"][0, 1])
    loss = sum_i / (B * M) + ALPHA_T_TO_I * sum_t / (B * K)
    kernel.last_results = res
    return np.array(loss, dtype=np.float32)


# revision 26
# speedup vs baseline: 1.0584x; 1.0220x over previous
"""Chamfer loss kernel for TRN2 (8 NeuronCores, data-parallel over batch).

Reference computation (per batch b):
  t = l2_normalize(tokens[b])      # (K=1024, D=128)
  i = l2_normalize(interests[b])   # (M=64,  D=128)
  dist[k,m] = sqrt(2 - 2*dot(t_k, i_m))   (unit vectors)
  loss = mean_bm(min_k dist) + 0.3 * mean_bk(min_m dist)

Design notes (per core, 64 batches; engine-balanced against the ~1.46us/batch
token-DMA floor):
  - interests pre-normalized once (phase 0, bn_stats for sum-of-squares),
    kept transposed in SBUF as bf16 iT_all.
  - per batch:
      DMA  tokens[b] -> t_all fp32 [128,(8n),128d]
      PE   8 transposes (fp32) -> psum; ACT evacuates psum -> tT bf16
      DVE  tq = tT*tT (bf16 2x mode)
      PE   8 dot matmuls   pdots[k,(n m)] = tT_n.T @ iT_b        (bf16)
      PE   8 ones-column matmuls sums[k,n] = tq_n.T @ ones  == sum_d t^2
           (lands token sum-of-squares directly in [k-partition, n] layout:
            no partition reduce, no DMA gather)
      ACT  tnrm = sqrt(sums) from psum;  DVE invt = 1/tnrm
      DVE  dn = pdots * invt  (fused normalize + psum evacuation, bf16)
      POOL st_t_all[:,b,:] = max_m dn   (deferred sqrt, once at the end)
      POOL m2 = max(dn[:,0:4,:], dn[:,4:8,:]);  DVE m3, nmax (bf16 tree)
      POOL partition-max nmax -> st_i chunk;  every 8 batches ACT applies
           sqrt(2-2x) to the chunk and POOL accumulates.
Host combines the 8 per-core partial sums.
"""

import numpy as np
from contextlib import ExitStack

import concourse.bass as bass
import concourse.bass_isa as bass_isa
import concourse.mybir as mybir
import concourse.tile as tile
from concourse import bacc
from concourse.bass_utils import run_bass_kernel_spmd

N_CORES = 8
B, K, M, D = 512, 1024, 64, 128
B_LOC = B // N_CORES          # 64 batches per core
KT = K // 128                 # 8 token tiles of [128, D] per batch
NG = B_LOC * M // 128         # 32 interest row-groups of 128
ALPHA_T_TO_I = 0.3
SI = 8                        # i-side sqrt staging (batches per chunk)
LAG = 5

F32 = mybir.dt.float32
BF16 = mybir.dt.bfloat16
AX = mybir.AxisListType
OP = mybir.AluOpType
ACT = mybir.ActivationFunctionType
RED = bass_isa.ReduceOp


def build(b_loc=B_LOC):
    assert b_loc % SI == 0
    nc = bacc.Bacc(
        "TRN2",
        target_bir_lowering=False,
        debug=False,
        num_devices=N_CORES,
    )
    tokens = nc.dram_tensor("tokens", [b_loc, K, D], F32, kind="ExternalInput").ap()
    interests = nc.dram_tensor(
        "interests", [b_loc, M, D], F32, kind="ExternalInput"
    ).ap()
    out = nc.dram_tensor("out", [1, 2], F32, kind="ExternalOutput").ap()

    with ExitStack() as ctx:
        tc = ctx.enter_context(tile.TileContext(nc))
        singles = ctx.enter_context(tc.tile_pool(name="singles", bufs=1))
        tok_pool = ctx.enter_context(tc.tile_pool(name="tok", bufs=4))
        tT_pool = ctx.enter_context(tc.tile_pool(name="tT", bufs=4))
        tq_pool = ctx.enter_context(tc.tile_pool(name="tq", bufs=4))
        dn_pool = ctx.enter_context(tc.tile_pool(name="dn", bufs=4))
        m2_pool = ctx.enter_context(tc.tile_pool(name="m2", bufs=4))
        small = ctx.enter_context(tc.tile_pool(name="small", bufs=16))
        stage = ctx.enter_context(tc.tile_pool(name="stage", bufs=3))
        p_tT = ctx.enter_context(tc.tile_pool(name="p_tT", bufs=3, space="PSUM"))
        p_dots = ctx.enter_context(tc.tile_pool(name="p_dots", bufs=2, space="PSUM"))
        p_sums = ctx.enter_context(tc.tile_pool(name="p_sums", bufs=1, space="PSUM"))

        identity = singles.tile([128, 128], F32)
        nc.gpsimd.memset(identity, 0.0)
        nc.gpsimd.affine_select(
            out=identity, in_=identity, compare_op=OP.not_equal, fill=1.0,
            base=0, pattern=[[-1, 128]], channel_multiplier=1,
        )
        ones_bf = singles.tile([128, 1], BF16)
        nc.vector.memset(ones_bf, 1.0)
        two = singles.tile([128, 1], F32)
        nc.vector.memset(two, 2.0)
        st_t_all = singles.tile([128, b_loc, KT], BF16)
        acc_i = singles.tile([128, SI * M], F32)
        nc.vector.memset(acc_i, 0.0)

        # ---------- phase 0: normalize + transpose all interests ----------
        # Emitted in 4 chunks, interleaved into the main pipeline so the
        # first token batches' DMA/transpose/square stages are not serialized
        # behind the whole interests preparation.
        i_flat = interests.rearrange("b m d -> (b m) d").rearrange(
            "(g p) d -> p g d", p=128
        )  # [128, NG, 128]
        i_all = singles.tile([128, NG, D], F32)
        i_n = singles.tile([128, NG, D], F32)
        iT_all = singles.tile([128, NG, 128], BF16)  # [d, (g, row)]
        # graded chunk sizes: tiny first chunks so the first batches' dot
        # matmuls are unblocked almost immediately
        CH_STARTS = [0, 2, 4, 8, 16, 24, 32]
        NCH = len(CH_STARTS) - 1

        def pA(c):  # interests chunk DMA
            sl = slice(CH_STARTS[c], CH_STARTS[c + 1])
            nc.sync.dma_start(out=i_all[:, sl], in_=i_flat[:, sl])

        def pB(c):  # sum-of-squares per interest row -> normalized i_n
            sl = slice(CH_STARTS[c], CH_STARTS[c + 1])
            gc = CH_STARTS[c + 1] - CH_STARTS[c]
            isq = tq_pool.tile([128, gc, D], BF16, tag="isq")
            nc.scalar.square(isq, i_all[:, sl])
            issq = small.tile([128, gc], F32, tag="issq")
            nc.vector.tensor_reduce(issq, isq, axis=AX.X, op=OP.add)
            inrm = small.tile([128, gc], F32, tag="inrm")
            nc.scalar.sqrt(inrm, issq)
            invi = small.tile([128, gc], F32, tag="invi")
            nc.vector.reciprocal(invi, inrm)
            nc.gpsimd.tensor_mul(
                i_n[:, sl], i_all[:, sl], invi.broadcast_to([128, gc, D])
            )

        def pC(c):  # transpose + evacuate chunk (sub-chunks of <=4 groups)
            lo, hi = CH_STARTS[c], CH_STARTS[c + 1]
            for cc in range(lo, hi, 4):
                w = min(4, hi - cc)
                piT = p_tT.tile([128, 4, 128], F32, tag="ptT")
                for j in range(w):
                    nc.tensor.transpose(
                        piT[:, j, :], i_n[:, cc + j, :], identity
                    )
                dst = iT_all[:, cc:cc + w, :].rearrange("p a b -> p (a b)")
                src = piT[:, :w, :].rearrange("p a b -> p (a b)")
                nc.scalar.copy(dst, src)

        def iT_of(b):
            return iT_all[:, b // 2, (b % 2) * M:(b % 2) * M + M]

        # ---------- software-pipelined main loop ----------
        # Post-matmul vector work is fused over batch PAIRS to amortize the
        # fixed per-op access latencies on DVE.  All free-axis reductions and
        # maxes are DVE-only (gpsimd has neither); Pool gets the elementwise
        # square's other half, the partition-max, and the accumulate adds.
        t_of, tT_of, pd_of, ps_of, iv_of = {}, {}, {}, {}, {}
        m2_of, sti_of = {}, {}

        def s0(b):  # token DMA
            t_all = tok_pool.tile([128, KT, D], F32)
            nc.sync.dma_start(
                out=t_all, in_=tokens[b].rearrange("(n p) d -> p n d", p=128)
            )
            t_of[b] = t_all

        def s1(b):  # transposes + evacuation (fp32 psum -> bf16 sbuf)
            t_all = t_of.pop(b)
            tT = tT_pool.tile([128, KT, 128], BF16, tag="tT")
            for h in range(2):
                ptT = p_tT.tile([128, KT // 2, 128], F32, tag="ptT")
                for j in range(KT // 2):
                    nc.tensor.transpose(
                        ptT[:, j, :], t_all[:, 4 * h + j, :], identity
                    )
                nc.scalar.copy(
                    tT[:, 4 * h:4 * h + 4, :].rearrange("p a b -> p (a b)"),
                    ptT.rearrange("p a b -> p (a b)"),
                )
            tT_of[b] = tT

        def s2(b):  # squares (DVE/Pool halves), dots, sum-of-squares columns
            tT = tT_of.pop(b)
            tq = tq_pool.tile([128, KT, 128], BF16, tag="tq")
            nc.vector.tensor_mul(
                tq[:, 0:4, :].rearrange("p a b -> p (a b)"),
                tT[:, 0:4, :].rearrange("p a b -> p (a b)"),
                tT[:, 0:4, :].rearrange("p a b -> p (a b)"),
            )
            nc.gpsimd.tensor_mul(
                tq[:, 4:8, :].rearrange("p a b -> p (a b)"),
                tT[:, 4:8, :].rearrange("p a b -> p (a b)"),
                tT[:, 4:8, :].rearrange("p a b -> p (a b)"),
            )
            if b % 2 == 0:
                pd2 = p_dots.tile([128, 2, KT, M], F32, tag="pd")
                ps2 = p_sums.tile([128, 2, KT], F32, tag="ps")
                pd_of[b // 2] = pd2
                ps_of[b // 2] = ps2
            else:
                pd2 = pd_of[b // 2]
                ps2 = ps_of[b // 2]
            h = b % 2
            for n in range(KT):
                nc.tensor.matmul(
                    ps2[:, h, n:n + 1], lhsT=tq[:, n, :], rhs=ones_bf,
                    start=True, stop=True,
                )
            iT = iT_of(b)
            for n in range(KT):
                nc.tensor.matmul(
                    pd2[:, h, n, :], lhsT=tT[:, n, :], rhs=iT,
                    start=True, stop=True,
                )

        def s2b(b):  # (folded back into s2)
            pass

        tn_of = {}

        def s3a(j):  # pair sqrt of sum-of-squares (ACT, reads psum)
            tnrm = small.tile([128, 2, KT], F32, tag="tnrm")
            nc.scalar.sqrt(tnrm, ps_of.pop(j))
            tn_of[j] = tnrm

        def s3(j):  # pair reciprocal -> invt
            invt = small.tile([128, 2, KT], F32, tag="invt")
            nc.vector.reciprocal(invt, tn_of.pop(j))
            iv_of[j] = invt

        q5_of, m2q_of = {}, {}

        def s4(j):  # normalize; m-tree levels 1-2; n-tree level 1  (pair j)
            pd2 = pd_of.pop(j)
            invt = iv_of.pop(j)
            q = j // 2
            h = j % 2
            if h == 0:
                q5_new = m2_pool.tile([128, 2, 2, KT, 16], BF16, tag="q5")
                m2q_new = m2_pool.tile([128, 2, 2, KT // 2, M], BF16, tag="m2q")
                q5_of[q] = q5_new
                m2q_of[q] = m2q_new
            dn = dn_pool.tile([128, 2, KT, M], BF16, tag="dn")
            nc.vector.tensor_mul(dn, pd2, invt.broadcast_to([128, 2, KT, M]))
            # t->i: per-token max over m, bf16 TT-max tree levels 64->32->16
            t32 = m2_pool.tile([128, 2, KT, 32], BF16, tag="t32")
            nc.vector.tensor_max(t32, dn[:, :, :, 0:32], dn[:, :, :, 32:64])
            nc.vector.tensor_max(
                q5_of[q][:, h], t32[:, :, :, 0:16], t32[:, :, :, 16:32]
            )
            # i->t: max over n, tree level 1 (8 -> 4)
            nc.vector.tensor_max(
                m2q_of[q][:, h], dn[:, :, 0:KT // 2, :], dn[:, :, KT // 2:KT, :]
            )

        def s5(q):  # finish both reductions for quad q (4 batches)
            b0 = 4 * q
            s2i = b0 % SI
            g = b0 // SI
            if s2i == 0:
                st_i_new = stage.tile([128, SI, M], BF16, tag="sti")
                sti_of[g] = st_i_new
            st_i = sti_of[g]
            # t->i tail: 16 -> 8 -> 4 -> 2 -> 1 over m
            q5 = q5_of.pop(q)
            t8 = small.tile([128, 2, 2, KT, 8], BF16, tag="t8")
            nc.vector.tensor_max(t8, q5[:, :, :, :, 0:8], q5[:, :, :, :, 8:16])
            t4 = small.tile([128, 2, 2, KT, 4], BF16, tag="t4")
            nc.vector.tensor_max(t4, t8[:, :, :, :, 0:4], t8[:, :, :, :, 4:8])
            t2 = small.tile([128, 2, 2, KT, 2], BF16, tag="t2")
            nc.vector.tensor_max(t2, t4[:, :, :, :, 0:2], t4[:, :, :, :, 2:4])
            nc.vector.tensor_max(
                st_t_all[:, b0:b0 + 4, :].rearrange(
                    "p (a b) (c o) -> p a b c o", a=2, o=1
                ),
                t2[:, :, :, :, 0:1], t2[:, :, :, :, 1:2],
            )
            # i->t tail: n-tree levels 2-3, then partition max
            m2q = m2q_of.pop(q)
            m3 = small.tile([128, 2, 2, 2, M], BF16, tag="m3")
            nc.vector.tensor_max(m3, m2q[:, :, :, 0:2, :], m2q[:, :, :, 2:4, :])
            nm2 = small.tile([128, 2, 2, M], BF16, tag="nm2")
            nc.vector.tensor_max(nm2, m3[:, :, :, 0, :], m3[:, :, :, 1, :])
            nc.gpsimd.partition_all_reduce(
                st_i[:, s2i:s2i + 4, :].rearrange("p a b -> p (a b)"),
                nm2.rearrange("p a b c -> p (a b c)"),
                channels=128, reduce_op=RED.max,
            )
            if s2i == SI - 4:
                del sti_of[g]
                di = stage.tile([128, SI * M], BF16, tag="di")
                nc.scalar.activation(
                    di, st_i.rearrange("p a b -> p (a b)"),
                    ACT.Sqrt, bias=two[:], scale=-2.0,
                )
                nc.gpsimd.tensor_add(acc_i, acc_i, di)

        nj = b_loc // 2
        nq = b_loc // 4
        for v in range(b_loc + 2 * LAG + 2):
            # interleaved phase-0 chunks
            if v < NCH:
                pA(v)
            if 1 <= v < NCH + 1:
                pB(v - 1)
            if 2 <= v < NCH + 2:
                pC(v - 2)
            # quad stages
            if v >= 8 and (v - 8) % 4 == 0 and (v - 8) // 4 < nq:
                s5((v - 8) // 4)
            # pair stages (recip must precede D4 in DVE program order)
            if v >= 5 and (v - 5) % 2 == 0 and (v - 5) // 2 < nj:
                s3((v - 5) // 2)
            if v >= 4 and (v - 4) % 2 == 1 and (v - 4) // 2 < nj:
                s4((v - 4) // 2)
            if v >= 4 and (v - 4) % 2 == 0 and (v - 4) // 2 < nj:
                s3a((v - 4) // 2)
            # batch stages
            if v >= 3 and v - 3 < b_loc:
                s2b(v - 3)
            if v >= 2 and v - 2 < b_loc:
                s2(v - 2)
            if v >= 1 and v - 1 < b_loc:
                s1(v - 1)
            if v < b_loc:
                s0(v)

        # ---------- final reductions ----------
        dt = singles.tile([128, b_loc * KT], BF16)
        nc.scalar.activation(
            dt, st_t_all.rearrange("p a b -> p (a b)"),
            ACT.Sqrt, bias=two[:], scale=-2.0,
        )
        red_t = singles.tile([128, 1], F32)
        nc.vector.tensor_reduce(red_t, dt, axis=AX.X, op=OP.add)
        rep_t = singles.tile([128, 1], F32)
        nc.gpsimd.partition_all_reduce(
            rep_t, red_t, channels=128, reduce_op=RED.add
        )
        red_i = singles.tile([128, 1], F32)
        nc.vector.tensor_reduce(red_i, acc_i, axis=AX.X, op=OP.add)
        out_sb = small.tile([1, 2], F32, tag="out_sb")
        nc.scalar.copy(out_sb[:, 0:1], rep_t[0:1, :])
        nc.scalar.copy(out_sb[:, 1:2], red_i[0:1, :])
        nc.sync.dma_start(out=out, in_=out_sb)

    nc.compile()
    return nc


_NC_CACHE = None


def _get_nc():
    global _NC_CACHE
    if _NC_CACHE is None:
        _NC_CACHE = build()
    return _NC_CACHE


def kernel(tokens: np.ndarray, interests: np.ndarray, _trace=False) -> np.ndarray:
    tokens = np.ascontiguousarray(tokens, dtype=np.float32)
    interests = np.ascontiguousarray(interests, dtype=np.float32)
    assert tokens.shape == (B, K, D) and interests.shape == (B, M, D)

    nc = _get_nc()
    in_maps = [
        {
            "tokens": tokens[c * B_LOC:(c + 1) * B_LOC],
            "interests": interests[c * B_LOC:(c + 1) * B_LOC],
        }
        for c in range(N_CORES)
    ]
    res = run_bass_kernel_spmd(
        nc, in_maps, core_ids=list(range(N_CORES)), trace=_trace
    )
    sum_t = 0.0  # sum over all (b, k) of min_m dist
    sum_i = 0.0  # sum over all (b, m) of min_k dist
    for r in res.results:
        sum_t += float(r["out"][0, 0])
        sum_i += float(r["out"][0, 1])
    loss = sum_i / (B * M) + ALPHA_T_TO_I * sum_t / (B * K)
    kernel.last_results = res
    return np.array(loss, dtype=np.float32)
